# revision 74
# baseline (speedup 1.0000x reference)
"""AttentionClustering kernel for Trainium2, 8 NeuronCores, data-parallel over batch.

Pipeline per core (one image, NCHW f32 in / f32 out):
  conv3x3(replicate pad) + relu  -> conv3x3(replicate pad) + relu -> 1x1 conv
  -> squared-distance logits vs 32 cluster centers -> softmax over clusters
  -> linear recombination with cluster_label.

Implementation notes:
  * Convs run as shifted matmuls accumulating in PSUM, fp16 inputs / f32 accum.
    q1 is stored twice in SBUF partitions (rows 64-127 shifted one image row)
    so the dy=0/dy=1 taps fuse into single K=128 matmuls.
  * conv2/conv3 pack two 2-row groups per PSUM bank (even group on partitions
    0-63, odd on 64-127); q2/q use the matching parity-packed layout. This
    halves the PSUM->SBUF copy count on ACT/DVE and doubles effective PSUM
    slots, which keeps the PE from micro-stalling (HAM stays warm).
  * softmax max-subtraction is algebraically unnecessary here: logits reduce
    (shift-invariance) to 2 q.mu - |mu|^2 < 0; |mu|^2 folds into the exp bias.
  * The per-pixel 1/sum runs on a [128,16] reshape (DRAM round-trip);
    DVE reciprocal is lane-parallel over partitions, so the matmul's [4,512]
    layout would be ~25x slower.
"""
import sys

sys.path.insert(0, "/opt/trn_rl_repo")

import numpy as np
import ml_dtypes

import concourse.bass as bass
import concourse.mybir as mybir
from concourse import bacc, bass_utils
from concourse.tile import TileContext

F32 = mybir.dt.float32
F16 = mybir.dt.float16
BF16 = mybir.dt.bfloat16

B, CIN, H, W = 8, 3, 256, 256
Q, NC, COUT = 64, 32, 64
R = 16          # output rows per strip
S = H // R      # strips
ACT_F = mybir.ActivationFunctionType
ALU = mybir.AluOpType

_cache = {}


def _build():
    nc = bacc.Bacc()
    xpad_t = nc.dram_tensor("xpad", (CIN, H + 2, W + 2), F16, kind="ExternalInput")
    w1c_t = nc.dram_tensor("w1c", (59, Q), F16, kind="ExternalInput")
    wa_t = nc.dram_tensor("wa", (128, 384), F16, kind="ExternalInput")
    mu2b_t = nc.dram_tensor("mu2b", (128, NC), F16, kind="ExternalInput")
    lb4_t = nc.dram_tensor("lb4", (128, COUT), BF16, kind="ExternalInput")
    ones_t = nc.dram_tensor("onesb", (128, 4), BF16, kind="ExternalInput")
    b1_t = nc.dram_tensor("b1c", (128, 1), F32, kind="ExternalInput")
    b2_t = nc.dram_tensor("b2c", (128, 1), F32, kind="ExternalInput")
    nmun_t = nc.dram_tensor("nmun", (128, 1), F32, kind="ExternalInput")
    dscr = nc.dram_tensor("dscr", (S, 2, 2048), F32, kind="Internal")
    rscr = nc.dram_tensor("rscr", (S, 8, 512), F32, kind="Internal")
    out_t = nc.dram_tensor("res", (COUT, H, W), F32, kind="ExternalOutput")

    with TileContext(nc) as tc:
        with (
            tc.tile_pool(name="consts", bufs=1) as cpool,
            tc.tile_pool(name="xcol", bufs=3) as xcol_pool,
            tc.tile_pool(name="q1p", bufs=3) as q1_pool,
            tc.tile_pool(name="q2", bufs=3) as q2_pool,
            tc.tile_pool(name="e4", bufs=10) as e_pool,
            tc.tile_pool(name="dr", bufs=8) as dr_pool,
            tc.tile_pool(name="rbc", bufs=7) as rbc_pool,
            tc.tile_pool(name="resf", bufs=6) as res_pool,
            tc.tile_pool(name="pc1", bufs=3, space="PSUM") as psum_c1,
            tc.tile_pool(name="c23", bufs=2, space="PSUM") as psum_c23,
            tc.tile_pool(name="psb", bufs=3, space="PSUM") as psum_sb,
        ):
            w1cT = cpool.tile([59, Q], F16)
            nc.sync.dma_start(w1cT[:, :], w1c_t[:, :])
            waT = cpool.tile([128, 384], F16)
            nc.sync.dma_start(waT[:, :], wa_t[:, :])
            mu2T = cpool.tile([128, NC], F16)
            # (b1T below is [128,1]: conv1's paired relu covers both halves)
            nc.sync.dma_start(mu2T[:, :], mu2b_t[:, :])
            lb4T = cpool.tile([128, COUT], BF16)
            nc.sync.dma_start(lb4T[:, :], lb4_t[:, :])
            onesT = cpool.tile([128, 4], BF16)
            nc.sync.dma_start(onesT[:, :], ones_t[:, :])
            b1T = cpool.tile([128, 1], F32)
            nc.sync.dma_start(b1T[:, :], b1_t[:, :])
            b2T = cpool.tile([128, 1], F32)
            nc.sync.dma_start(b2T[:, :], b2_t[:, :])
            nmunT = cpool.tile([128, 1], F32)
            nc.sync.dma_start(nmunT[:, :], nmun_t[:, :])

            e4s, dres, rbcs = {}, {}, {}

            def sb1(qt, s):
                """logits, exp, per-pixel sum, and the 1/d DRAM round-trip kick."""
                for Qd in range(2):
                    ps = psum_sb.tile([128, 512], F32, tag="sb")
                    for j in range(4):
                        g = 4 * Qd + j
                        h = 64 * (g % 2)
                        nc.tensor.matmul(
                            ps[32 * j : 32 * j + 32, :],
                            mu2T[h : h + 64, :],
                            qt[h : h + 64, 512 * (g // 2) : 512 * (g // 2 + 1)],
                            start=True, stop=True, tile_position=(h, 32 * j),
                        )
                    e4 = e_pool.tile([128, 512], BF16)
                    e4s[(s, Qd)] = e4
                    nc.scalar.activation(e4[:, :], ps[:, :], ACT_F.Exp,
                                         bias=nmunT[:, :], scale=1.0)
                    pd = psum_sb.tile([128, 512], F32, tag="sb")
                    nc.tensor.matmul(pd[0:4, :], onesT[:, :], e4[:, :],
                                     start=True, stop=True)
                    d4s = dr_pool.tile([4, 512], F32, tag="d4s")
                    nc.vector.tensor_copy(d4s[:, :], pd[0:4, :])
                    # store group g at offset (g%2)*1024 + (g//2)*512 so the
                    # rbc broadcast read below is a 3-dim AP
                    nc.sync.dma_start(
                        bass.AP(tensor=dscr, offset=(s * 2 + Qd) * 2048,
                                ap=[[512, 2], [1024, 2], [1, 512]]),
                        d4s[:, :])
                    # reload as [128,16] so the reciprocal is lane-parallel
                    dre = dr_pool.tile([128, 16], F32, tag="dre")
                    dres[(s, Qd)] = dre
                    nc.sync.dma_start(
                        dre[:, :],
                        bass.AP(tensor=dscr, offset=(s * 2 + Qd) * 2048,
                                ap=[[16, 128], [1, 16]]),
                    )

            def sb2(s):
                """reciprocal (DMA long since landed) + partition-broadcast kick."""
                for Qd in range(2):
                    rre = dr_pool.tile([128, 16], F32, tag="rre")
                    nc.vector.reciprocal(rre[:, :], dres.pop((s, Qd))[:, :])
                    nc.sync.dma_start(
                        bass.AP(tensor=rscr, offset=(s * 8 + 4 * Qd) * 512,
                                ap=[[16, 128], [1, 16]]),
                        rre[:, :],
                    )
                    rbc = rbc_pool.tile([128, 1024], F32)
                    rbcs[(s, Qd)] = rbc
                    # two half-broadcasts on different rings: separate source
                    # regions parallelize (stride-0 reads serialize per region)
                    nc.gpsimd.dma_start(
                        rbc[0:64, :],
                        bass.AP(tensor=rscr, offset=(s * 8 + 4 * Qd) * 512,
                                ap=[[0, 64], [1, 1024]]),
                    )
                    nc.sync.dma_start(
                        rbc[64:128, :],
                        bass.AP(tensor=rscr, offset=(s * 8 + 4 * Qd) * 512 + 1024,
                                ap=[[0, 64], [1, 1024]]),
                    )

            def sb3(s):
                """label recombination, normalization, store."""
                r0 = R * s
                for Qd in range(2):
                    e4 = e4s.pop((s, Qd))
                    for pp in range(2):
                        pr = psum_sb.tile([128, 512], F32, tag="sb")
                        for k in range(2):
                            j = 2 * pp + k
                            nc.tensor.matmul(
                                pr[64 * k : 64 * k + 64, :],
                                lb4T[32 * j : 32 * j + 32, :],
                                e4[32 * j : 32 * j + 32, :],
                                start=True, stop=True,
                                tile_position=(32 * j, 64 * k),
                            )
                        resf = res_pool.tile([128, 512], F32)
                        rbc = rbcs[(s, Qd)]
                        nc.vector.tensor_tensor(resf[:, :], pr[:, :],
                                                rbc[:, 512 * pp : 512 * pp + 512],
                                                op=ALU.mult)
                        if pp == 1:
                            rbcs.pop((s, Qd))
                        g0 = 4 * Qd + 2 * pp
                        row = r0 + 2 * g0
                        nc.scalar.dma_start(
                            out_t[:, row : row + 2, :],
                            resf[0:64, :].rearrange("p (r c) -> p r c", r=2),
                        )
                        nc.scalar.dma_start(
                            out_t[:, row + 2 : row + 4, :],
                            resf[64:128, :].rearrange("p (r c) -> p r c", r=2),
                        )

            xcols = {}

            def load_xcol(s):
                if s == 0:
                    A = 0
                elif s == S - 1:
                    A = H - 18
                else:
                    A = R * s - 1
                xcol = xcol_pool.tile([59, 18, 256], F16)
                xcols[s] = xcol
                dma_eng = [nc.sync, nc.scalar, nc.gpsimd]
                for dy in range(3):
                    for dx in range(3):
                        p = (3 * dy + dx) * 3
                        dma_eng[dy].dma_start(
                            xcol[p : p + 3, :, :],
                            xpad_t[:, A + dy : A + dy + 18, dx : dx + 256],
                        )
                # duplicate onto partitions 32-58 so conv1's upper matmul
                # lives in row-group 1 (concurrent with the lower, rg0)
                nc.scalar.dma_start(xcol[32:59, :, :], xcol[0:27, :, :])

            q1ps = {}

            def conv1_part(s2, part):
                """Emit conv1 groups G in [3*part, 3*part+3) for strip s2.

                Each group is a PAIR of col-tiled matmuls: lower half of the
                PSUM tile gets output rows {2G, 2G+1}; the upper half (col
                group 64-127, same weights/xcol partitions) reads xcol ONE ROW
                DOWN, directly materializing the row-shifted duplicate that
                conv2's dy-packing needs.  One [128,512] relu covers both, and
                the old 594KB shift DMA disappears."""
                woff = 1 if s2 == 0 else 0
                if part == 0:
                    q1p_t = q1_pool.tile([128, 19, 258], F16)
                    q1ps[s2] = q1p_t
                q1p = q1ps[s2]
                xcol = xcols[s2]

                def pads(k):
                    # col replicate-pads on idle gpsimd; the two halves have
                    # different written-row coverage in the last chunk
                    r = 6 * k
                    nl = 7 if k < 2 else (7 if s2 == S - 1 else 6)
                    nu = 7 if k < 2 else 5
                    for p0, p1, n in ((0, 64, nl), (64, 128, nu)):
                        nc.gpsimd.tensor_copy(q1p[p0:p1, r : r + n, 0:1],
                                              q1p[p0:p1, r : r + n, 1:2])
                        nc.gpsimd.tensor_copy(q1p[p0:p1, r : r + n, 257:258],
                                              q1p[p0:p1, r : r + n, 256:257])

                for G in range(3 * part, 3 * part + 3):
                    a = 2 * G + woff
                    pc1 = psum_c1.tile([128, 512], F32, tag="pc1")
                    nc.tensor.matmul(
                        pc1[0:64, :], w1cT[0:27, :],
                        xcol[0:27, 2 * G : 2 * G + 2, :],
                        start=True, stop=True, tile_position=(0, 0),
                        skip_group_check=True,
                    )
                    if G < 8:
                        nc.tensor.matmul(
                            pc1[64:128, :], w1cT[32:59, :],
                            xcol[32:59, 2 * G + 1 : 2 * G + 3, :],
                            start=True, stop=True, tile_position=(32, 64),
                            skip_group_check=True,
                        )
                        dst = q1p[:, a : a + 2, 1:257]
                        src = pc1[:, :].rearrange("p (r c) -> p r c", r=2)
                        if G % 2 == 0:
                            nc.scalar.activation(dst, src, ACT_F.Relu,
                                                 bias=b1T[:, :], scale=1.0)
                        else:
                            nc.vector.tensor_scalar(dst, src, b1T[:, :], 0.0,
                                                    ALU.add, ALU.max)
                    else:
                        # last pair: upper half only has one in-range row
                        nc.tensor.matmul(
                            pc1[64:128, 0:256], w1cT[32:59, :],
                            xcol[32:59, 17:18, :],
                            start=True, stop=True, tile_position=(32, 64),
                            skip_group_check=True,
                        )
                        nc.scalar.activation(
                            q1p[0:64, a : a + 2, 1:257],
                            pc1[0:64, :].rearrange("p (r c) -> p r c", r=2),
                            ACT_F.Relu, bias=b1T[0:64, :], scale=1.0)
                        nc.vector.tensor_scalar(
                            q1p[64:128, a : a + 1, 1:257],
                            pc1[64:128, 0:256].rearrange("p (r c) -> p r c", r=1),
                            b1T[64:128, :], 0.0, ALU.add, ALU.max)
                    if G == 0 and s2 == 0:
                        nc.vector.tensor_copy(q1p[0:64, 0:1, 1:257],
                                              q1p[0:64, 1:2, 1:257])
                    if G == 3:
                        if s2 == 0:
                            # upper idx 0 (content row 0) via one small DMA
                            nc.scalar.dma_start(q1p[64:128, 0:1, 1:257],
                                                q1p[0:64, 1:2, 1:257])
                        pads(0)
                    elif G == 6:
                        pads(1)
                    elif G == 8:
                        if s2 == S - 1:
                            nc.vector.tensor_copy(q1p[0:64, 18:19, 1:257],
                                                  q1p[0:64, 17:18, 1:257])
                        pads(2)
                        if s2 == S - 1:
                            # upper idx 17 = content row 18 = replicate row 17
                            nc.gpsimd.tensor_copy(q1p[64:128, 17:18, :],
                                                  q1p[64:128, 16:17, :])
                        xcols.pop(s2)

            pending = None
            for s in range(S):
                r0 = R * s
                rb = 1 if s == S - 1 else 0

                # conv1(s+1) is emitted inside this strip's conv2 (below), so
                # the q1p shift chunks land long before conv2(s+1) needs them.
                if s == 0:
                    load_xcol(0)
                    load_xcol(1)
                    for part in range(3):
                        conv1_part(0, part)
                if s + 2 < S:
                    load_xcol(s + 2)

                # pipelined stage B: sb1 one strip back, sb2 two, sb3 three
                # (deep enough that recip/rbc DMA chains are pre-satisfied and
                # never head-of-line-block the DVE/ACT FIFOs)
                if pending is not None:
                    sb1(*pending)

                # ---- conv2: two column-chains (gh=0 cols 0-63, gh=1 cols
                # 64-127) interleaved so they run concurrently on the PE ----
                q1p = q1ps[s]
                q2t = q2_pool.tile([128, 2048], F16)
                for pi in range(4):
                    pc2 = psum_c23.tile([128, 512], F32, tag="c23")
                    for dx in range(3):
                        for gh in range(2):
                            g = 2 * pi + gh
                            h = 64 * gh
                            nc.tensor.matmul(
                                pc2[h : h + 64, :], waT[:, 64 * dx : 64 * dx + 64],
                                q1p[:, rb + 2 * g : rb + 2 * g + 2, dx : dx + 256],
                                start=(dx == 0), stop=False,
                                tile_position=(0, h), skip_group_check=True,
                            )
                    for dx in range(3):
                        for gh in range(2):
                            g = 2 * pi + gh
                            h = 64 * gh
                            nc.tensor.matmul(
                                pc2[h : h + 64, :],
                                waT[64:128, 192 + 64 * dx : 256 + 64 * dx],
                                q1p[64:128, rb + 2 * g + 1 : rb + 2 * g + 3, dx : dx + 256],
                                start=False, stop=(dx == 2),
                                tile_position=(64, h), skip_group_check=True,
                            )
                    dst = q2t[:, 512 * pi : 512 * (pi + 1)]
                    if pi % 2 == 0:
                        nc.scalar.activation(dst, pc2[:, :], ACT_F.Relu,
                                             bias=b2T[:, :], scale=1.0)
                    else:
                        nc.vector.tensor_scalar(dst, pc2[:, :], b2T[:, :], 0.0,
                                                ALU.add, ALU.max)
                    if pi == 1 and s >= 3:
                        sb3(s - 3)
                    if pi == 2 and s >= 2:
                        sb2(s - 2)
                    if pi < 3 and s + 1 < S:
                        conv1_part(s + 1, pi)

                q1ps.pop(s)
                pending = (q2t, s)

            sb2(S - 2)
            sb1(*pending)
            sb2(S - 1)
            sb3(S - 3)
            sb3(S - 2)
            sb3(S - 1)
    nc.finalize()
    return nc


def _prep_inputs(x, w1, b1, w2, b2, w3, b3, cluster_mu, cluster_label):
    f16 = np.float16
    bf16 = ml_dtypes.bfloat16
    xpad = np.pad(x, ((0, 0), (0, 0), (1, 1), (1, 1)), mode="edge").astype(f16)
    w1c27 = w1.transpose(2, 3, 1, 0).reshape(27, Q).astype(f16)
    w1c = np.zeros((59, Q), f16)
    w1c[0:27] = w1c27
    w1c[32:59] = w1c27
    # wa: [128, 384]; cols 0-191: dy=0 (rows 0-63) / dy=1 (rows 64-127) taps
    #     cols 192-383: dy=2 taps on rows 64-127
    wa = np.zeros((128, 384), f16)
    for dx in range(3):
        wa[0:64, 64 * dx : 64 * dx + 64] = w2[:, :, 0, dx].T
        wa[64:128, 64 * dx : 64 * dx + 64] = w2[:, :, 1, dx].T
        wa[64:128, 192 + 64 * dx : 256 + 64 * dx] = w2[:, :, 2, dx].T
    # conv3 folded into the cluster dot: 2 mu.(W3 q2 + b3) - |mu|^2
    #   = (2 mu W3).q2 + (2 mu.b3 - |mu|^2)
    w3r = w3.reshape(Q, Q).astype(np.float64)
    mu = cluster_mu.reshape(NC, Q).astype(np.float64)
    m2 = mu @ w3r                                  # (NC, Q) over q2 channels
    mu2 = (2.0 * m2).T.astype(f16)
    mu2b = np.ascontiguousarray(np.tile(mu2, (2, 1)))
    lb4 = np.tile(np.ascontiguousarray(cluster_label.T), (4, 1)).astype(bf16)
    onesb = np.zeros((128, 4), bf16)
    for j in range(4):
        onesb[32 * j : 32 * j + 32, j] = 1
    ebias = 2.0 * (mu @ b3.astype(np.float64)) - np.sum(mu * mu, axis=1)
    nmun = np.tile(ebias, 4).reshape(128, 1).astype(np.float32)
    shared = {
        "w1c": w1c, "wa": wa, "mu2b": mu2b, "lb4": lb4,
        "onesb": onesb,
        "b1c": np.tile(b1, 2).reshape(128, 1).astype(np.float32),
        "b2c": np.tile(b2, 2).reshape(128, 1).astype(np.float32),
        "nmun": nmun,
    }
    return [{"xpad": np.ascontiguousarray(xpad[b]), **shared} for b in range(B)]


def run(inputs, trace=False, **trace_kwargs):
    """Build (cached), run on 8 cores, return (output, BassKernelResults)."""
    if "nc" not in _cache:
        _cache["nc"] = _build()
    in_maps = _prep_inputs(**{k: np.asarray(v) for k, v in inputs.items()})
    res = bass_utils.run_bass_kernel_spmd(
        _cache["nc"], in_maps, core_ids=list(range(B)), trace=trace, **trace_kwargs
    )
    out = np.stack([res.results[b]["res"] for b in range(B)]).astype(np.float32)
    return out, res


def kernel(**inputs):
    out, _ = run(inputs)
    return out



# revision 75
# speedup vs baseline: 1.0029x; 1.0029x over previous
"""AttentionClustering kernel for Trainium2, 8 NeuronCores, data-parallel over batch.

Pipeline per core (one image, NCHW f32 in / f32 out):
  conv3x3(replicate pad) + relu  -> conv3x3(replicate pad) + relu -> 1x1 conv
  -> squared-distance logits vs 32 cluster centers -> softmax over clusters
  -> linear recombination with cluster_label.

Implementation notes:
  * Convs run as shifted matmuls accumulating in PSUM, fp16 inputs / f32 accum.
    q1 is stored twice in SBUF partitions (rows 64-127 shifted one image row)
    so the dy=0/dy=1 taps fuse into single K=128 matmuls.
  * conv2/conv3 pack two 2-row groups per PSUM bank (even group on partitions
    0-63, odd on 64-127); q2/q use the matching parity-packed layout. This
    halves the PSUM->SBUF copy count on ACT/DVE and doubles effective PSUM
    slots, which keeps the PE from micro-stalling (HAM stays warm).
  * softmax max-subtraction is algebraically unnecessary here: logits reduce
    (shift-invariance) to 2 q.mu - |mu|^2 < 0; |mu|^2 folds into the exp bias.
  * The per-pixel 1/sum runs on a [128,16] reshape (DRAM round-trip);
    DVE reciprocal is lane-parallel over partitions, so the matmul's [4,512]
    layout would be ~25x slower.
"""
import sys

sys.path.insert(0, "/opt/trn_rl_repo")

import numpy as np
import ml_dtypes

import concourse.bass as bass
import concourse.mybir as mybir
from concourse import bacc, bass_utils
from concourse.tile import TileContext

F32 = mybir.dt.float32
F16 = mybir.dt.float16
BF16 = mybir.dt.bfloat16

B, CIN, H, W = 8, 3, 256, 256
Q, NC, COUT = 64, 32, 64
R = 16          # output rows per strip
S = H // R      # strips
ACT_F = mybir.ActivationFunctionType
ALU = mybir.AluOpType

_cache = {}


def _build():
    nc = bacc.Bacc()
    xpad_t = nc.dram_tensor("xpad", (CIN, H + 2, W + 2), F16, kind="ExternalInput")
    w1c_t = nc.dram_tensor("w1c", (59, Q), F16, kind="ExternalInput")
    wa_t = nc.dram_tensor("wa", (128, 384), F16, kind="ExternalInput")
    mu2b_t = nc.dram_tensor("mu2b", (128, NC), F16, kind="ExternalInput")
    lb4_t = nc.dram_tensor("lb4", (128, COUT), BF16, kind="ExternalInput")
    ones_t = nc.dram_tensor("onesb", (128, 4), BF16, kind="ExternalInput")
    b1_t = nc.dram_tensor("b1c", (128, 1), F32, kind="ExternalInput")
    b2_t = nc.dram_tensor("b2c", (128, 1), F32, kind="ExternalInput")
    nmun_t = nc.dram_tensor("nmun", (128, 1), F32, kind="ExternalInput")
    dscr = nc.dram_tensor("dscr", (S, 2, 2048), F32, kind="Internal")
    rscr = nc.dram_tensor("rscr", (S, 8, 512), F32, kind="Internal")
    out_t = nc.dram_tensor("res", (COUT, H, W), F32, kind="ExternalOutput")

    with TileContext(nc) as tc:
        with (
            tc.tile_pool(name="consts", bufs=1) as cpool,
            tc.tile_pool(name="xcol", bufs=3) as xcol_pool,
            tc.tile_pool(name="q1p", bufs=3) as q1_pool,
            tc.tile_pool(name="q2", bufs=3) as q2_pool,
            tc.tile_pool(name="e4", bufs=8) as e_pool,
            tc.tile_pool(name="dr", bufs=6) as dr_pool,
            tc.tile_pool(name="rbc", bufs=5) as rbc_pool,
            tc.tile_pool(name="resf", bufs=4) as res_pool,
            tc.tile_pool(name="pc1", bufs=3, space="PSUM") as psum_c1,
            tc.tile_pool(name="c23", bufs=2, space="PSUM") as psum_c23,
            tc.tile_pool(name="psb", bufs=3, space="PSUM") as psum_sb,
        ):
            w1cT = cpool.tile([59, Q], F16)
            nc.sync.dma_start(w1cT[:, :], w1c_t[:, :])
            waT = cpool.tile([128, 384], F16)
            nc.sync.dma_start(waT[:, :], wa_t[:, :])
            mu2T = cpool.tile([128, NC], F16)
            # (b1T below is [128,1]: conv1's paired relu covers both halves)
            nc.sync.dma_start(mu2T[:, :], mu2b_t[:, :])
            lb4T = cpool.tile([128, COUT], BF16)
            nc.sync.dma_start(lb4T[:, :], lb4_t[:, :])
            onesT = cpool.tile([128, 4], BF16)
            nc.sync.dma_start(onesT[:, :], ones_t[:, :])
            b1T = cpool.tile([128, 1], F32)
            nc.sync.dma_start(b1T[:, :], b1_t[:, :])
            b2T = cpool.tile([128, 1], F32)
            nc.sync.dma_start(b2T[:, :], b2_t[:, :])
            nmunT = cpool.tile([128, 1], F32)
            nc.sync.dma_start(nmunT[:, :], nmun_t[:, :])

            e4s, dres, rbcs = {}, {}, {}

            def sb1(qt, s):
                """logits, exp, per-pixel sum, and the 1/d DRAM round-trip kick."""
                for Qd in range(2):
                    ps = psum_sb.tile([128, 512], F32, tag="sb")
                    for j in range(4):
                        g = 4 * Qd + j
                        h = 64 * (g % 2)
                        nc.tensor.matmul(
                            ps[32 * j : 32 * j + 32, :],
                            mu2T[h : h + 64, :],
                            qt[h : h + 64, 512 * (g // 2) : 512 * (g // 2 + 1)],
                            start=True, stop=True, tile_position=(h, 32 * j),
                        )
                    e4 = e_pool.tile([128, 512], BF16)
                    e4s[(s, Qd)] = e4
                    nc.scalar.activation(e4[:, :], ps[:, :], ACT_F.Exp,
                                         bias=nmunT[:, :], scale=1.0)
                    pd = psum_sb.tile([128, 512], F32, tag="sb")
                    nc.tensor.matmul(pd[0:4, :], onesT[:, :], e4[:, :],
                                     start=True, stop=True)
                    d4s = dr_pool.tile([4, 512], F32, tag="d4s")
                    nc.vector.tensor_copy(d4s[:, :], pd[0:4, :])
                    # store group g at offset (g%2)*1024 + (g//2)*512 so the
                    # rbc broadcast read below is a 3-dim AP
                    nc.sync.dma_start(
                        bass.AP(tensor=dscr, offset=(s * 2 + Qd) * 2048,
                                ap=[[512, 2], [1024, 2], [1, 512]]),
                        d4s[:, :])
                    # reload as [128,16] so the reciprocal is lane-parallel
                    dre = dr_pool.tile([128, 16], F32, tag="dre")
                    dres[(s, Qd)] = dre
                    nc.sync.dma_start(
                        dre[:, :],
                        bass.AP(tensor=dscr, offset=(s * 2 + Qd) * 2048,
                                ap=[[16, 128], [1, 16]]),
                    )

            def sb2(s):
                """reciprocal (DMA long since landed) + partition-broadcast kick."""
                for Qd in range(2):
                    rre = dr_pool.tile([128, 16], F32, tag="rre")
                    nc.vector.reciprocal(rre[:, :], dres.pop((s, Qd))[:, :])
                    nc.sync.dma_start(
                        bass.AP(tensor=rscr, offset=(s * 8 + 4 * Qd) * 512,
                                ap=[[16, 128], [1, 16]]),
                        rre[:, :],
                    )
                    rbc = rbc_pool.tile([128, 1024], F32)
                    rbcs[(s, Qd)] = rbc
                    # two half-broadcasts on different rings: separate source
                    # regions parallelize (stride-0 reads serialize per region)
                    nc.gpsimd.dma_start(
                        rbc[0:64, :],
                        bass.AP(tensor=rscr, offset=(s * 8 + 4 * Qd) * 512,
                                ap=[[0, 64], [1, 1024]]),
                    )
                    nc.sync.dma_start(
                        rbc[64:128, :],
                        bass.AP(tensor=rscr, offset=(s * 8 + 4 * Qd) * 512 + 1024,
                                ap=[[0, 64], [1, 1024]]),
                    )

            def sb3(s):
                """label recombination, normalization, store."""
                r0 = R * s
                for Qd in range(2):
                    e4 = e4s.pop((s, Qd))
                    for pp in range(2):
                        pr = psum_sb.tile([128, 512], F32, tag="sb")
                        for k in range(2):
                            j = 2 * pp + k
                            nc.tensor.matmul(
                                pr[64 * k : 64 * k + 64, :],
                                lb4T[32 * j : 32 * j + 32, :],
                                e4[32 * j : 32 * j + 32, :],
                                start=True, stop=True,
                                tile_position=(32 * j, 64 * k),
                            )
                        resf = res_pool.tile([128, 512], F32)
                        rbc = rbcs[(s, Qd)]
                        nc.vector.tensor_tensor(resf[:, :], pr[:, :],
                                                rbc[:, 512 * pp : 512 * pp + 512],
                                                op=ALU.mult)
                        if pp == 1:
                            rbcs.pop((s, Qd))
                        g0 = 4 * Qd + 2 * pp
                        row = r0 + 2 * g0
                        nc.scalar.dma_start(
                            out_t[:, row : row + 2, :],
                            resf[0:64, :].rearrange("p (r c) -> p r c", r=2),
                        )
                        nc.scalar.dma_start(
                            out_t[:, row + 2 : row + 4, :],
                            resf[64:128, :].rearrange("p (r c) -> p r c", r=2),
                        )

            xcols = {}

            def load_xcol(s):
                if s == 0:
                    A = 0
                elif s == S - 1:
                    A = H - 18
                else:
                    A = R * s - 1
                xcol = xcol_pool.tile([59, 18, 256], F16)
                xcols[s] = xcol
                dma_eng = [nc.sync, nc.scalar, nc.gpsimd]
                for dy in range(3):
                    for dx in range(3):
                        p = (3 * dy + dx) * 3
                        dma_eng[dy].dma_start(
                            xcol[p : p + 3, :, :],
                            xpad_t[:, A + dy : A + dy + 18, dx : dx + 256],
                        )
                # duplicate onto partitions 32-58 so conv1's upper matmul
                # lives in row-group 1 (concurrent with the lower, rg0)
                nc.scalar.dma_start(xcol[32:59, :, :], xcol[0:27, :, :])

            q1ps = {}

            def conv1_part(s2, part):
                """Emit conv1 groups G in [3*part, 3*part+3) for strip s2.

                Each group is a PAIR of col-tiled matmuls: lower half of the
                PSUM tile gets output rows {2G, 2G+1}; the upper half (col
                group 64-127, same weights/xcol partitions) reads xcol ONE ROW
                DOWN, directly materializing the row-shifted duplicate that
                conv2's dy-packing needs.  One [128,512] relu covers both, and
                the old 594KB shift DMA disappears."""
                woff = 1 if s2 == 0 else 0
                if part == 0:
                    q1p_t = q1_pool.tile([128, 19, 258], F16)
                    q1ps[s2] = q1p_t
                q1p = q1ps[s2]
                xcol = xcols[s2]

                def pads(k):
                    # col replicate-pads on idle gpsimd; the two halves have
                    # different written-row coverage in the last chunk
                    r = 6 * k
                    nl = 7 if k < 2 else (7 if s2 == S - 1 else 6)
                    nu = 7 if k < 2 else 5
                    for p0, p1, n in ((0, 64, nl), (64, 128, nu)):
                        nc.gpsimd.tensor_copy(q1p[p0:p1, r : r + n, 0:1],
                                              q1p[p0:p1, r : r + n, 1:2])
                        nc.gpsimd.tensor_copy(q1p[p0:p1, r : r + n, 257:258],
                                              q1p[p0:p1, r : r + n, 256:257])

                for G in range(3 * part, 3 * part + 3):
                    a = 2 * G + woff
                    pc1 = psum_c1.tile([128, 512], F32, tag="pc1")
                    nc.tensor.matmul(
                        pc1[0:64, :], w1cT[0:27, :],
                        xcol[0:27, 2 * G : 2 * G + 2, :],
                        start=True, stop=True, tile_position=(0, 0),
                        skip_group_check=True,
                    )
                    if G < 8:
                        nc.tensor.matmul(
                            pc1[64:128, :], w1cT[32:59, :],
                            xcol[32:59, 2 * G + 1 : 2 * G + 3, :],
                            start=True, stop=True, tile_position=(32, 64),
                            skip_group_check=True,
                        )
                        dst = q1p[:, a : a + 2, 1:257]
                        src = pc1[:, :].rearrange("p (r c) -> p r c", r=2)
                        if G % 2 == 0:
                            nc.scalar.activation(dst, src, ACT_F.Relu,
                                                 bias=b1T[:, :], scale=1.0)
                        else:
                            nc.vector.tensor_scalar(dst, src, b1T[:, :], 0.0,
                                                    ALU.add, ALU.max)
                    else:
                        # last pair: upper half only has one in-range row
                        nc.tensor.matmul(
                            pc1[64:128, 0:256], w1cT[32:59, :],
                            xcol[32:59, 17:18, :],
                            start=True, stop=True, tile_position=(32, 64),
                            skip_group_check=True,
                        )
                        nc.scalar.activation(
                            q1p[0:64, a : a + 2, 1:257],
                            pc1[0:64, :].rearrange("p (r c) -> p r c", r=2),
                            ACT_F.Relu, bias=b1T[0:64, :], scale=1.0)
                        nc.vector.tensor_scalar(
                            q1p[64:128, a : a + 1, 1:257],
                            pc1[64:128, 0:256].rearrange("p (r c) -> p r c", r=1),
                            b1T[64:128, :], 0.0, ALU.add, ALU.max)
                    if G == 0 and s2 == 0:
                        nc.vector.tensor_copy(q1p[0:64, 0:1, 1:257],
                                              q1p[0:64, 1:2, 1:257])
                    if G == 3:
                        if s2 == 0:
                            # upper idx 0 (content row 0) via one small DMA
                            nc.scalar.dma_start(q1p[64:128, 0:1, 1:257],
                                                q1p[0:64, 1:2, 1:257])
                        pads(0)
                    elif G == 6:
                        pads(1)
                    elif G == 8:
                        if s2 == S - 1:
                            nc.vector.tensor_copy(q1p[0:64, 18:19, 1:257],
                                                  q1p[0:64, 17:18, 1:257])
                        pads(2)
                        if s2 == S - 1:
                            # upper idx 17 = content row 18 = replicate row 17
                            nc.gpsimd.tensor_copy(q1p[64:128, 17:18, :],
                                                  q1p[64:128, 16:17, :])
                        xcols.pop(s2)

            pending = None
            for s in range(S):
                r0 = R * s
                rb = 1 if s == S - 1 else 0

                # conv1(s+1) is emitted inside this strip's conv2 (below), so
                # the q1p shift chunks land long before conv2(s+1) needs them.
                if s == 0:
                    load_xcol(0)
                    load_xcol(1)
                    for part in range(3):
                        conv1_part(0, part)
                if s + 2 < S:
                    load_xcol(s + 2)

                # pipelined stage B: sb1 one strip back, sb2 two, sb3 three
                # (deep enough that recip/rbc DMA chains are pre-satisfied and
                # never head-of-line-block the DVE/ACT FIFOs)
                if pending is not None:
                    sb1(*pending)

                # ---- conv2: two column-chains (gh=0 cols 0-63, gh=1 cols
                # 64-127) interleaved so they run concurrently on the PE ----
                q1p = q1ps[s]
                q2t = q2_pool.tile([128, 2048], F16)
                for pi in range(4):
                    pc2 = psum_c23.tile([128, 512], F32, tag="c23")
                    for dx in range(3):
                        for gh in range(2):
                            g = 2 * pi + gh
                            h = 64 * gh
                            nc.tensor.matmul(
                                pc2[h : h + 64, :], waT[:, 64 * dx : 64 * dx + 64],
                                q1p[:, rb + 2 * g : rb + 2 * g + 2, dx : dx + 256],
                                start=(dx == 0), stop=False,
                                tile_position=(0, h), skip_group_check=True,
                            )
                    for dx in range(3):
                        for gh in range(2):
                            g = 2 * pi + gh
                            h = 64 * gh
                            nc.tensor.matmul(
                                pc2[h : h + 64, :],
                                waT[64:128, 192 + 64 * dx : 256 + 64 * dx],
                                q1p[64:128, rb + 2 * g + 1 : rb + 2 * g + 3, dx : dx + 256],
                                start=False, stop=(dx == 2),
                                tile_position=(64, h), skip_group_check=True,
                            )
                    dst = q2t[:, 512 * pi : 512 * (pi + 1)]
                    if pi % 2 == 0:
                        nc.scalar.activation(dst, pc2[:, :], ACT_F.Relu,
                                             bias=b2T[:, :], scale=1.0)
                    else:
                        nc.vector.tensor_scalar(dst, pc2[:, :], b2T[:, :], 0.0,
                                                ALU.add, ALU.max)
                    if pi == 1 and s >= 3:
                        sb3(s - 3)
                    if pi == 2 and s >= 2:
                        sb2(s - 2)
                    if pi < 3 and s + 1 < S:
                        conv1_part(s + 1, pi)

                q1ps.pop(s)
                pending = (q2t, s)

            sb2(S - 2)
            sb1(*pending)
            sb2(S - 1)
            sb3(S - 3)
            sb3(S - 2)
            sb3(S - 1)
    nc.finalize()
    return nc


def _prep_inputs(x, w1, b1, w2, b2, w3, b3, cluster_mu, cluster_label):
    f16 = np.float16
    bf16 = ml_dtypes.bfloat16
    xpad = np.pad(x, ((0, 0), (0, 0), (1, 1), (1, 1)), mode="edge").astype(f16)
    w1c27 = w1.transpose(2, 3, 1, 0).reshape(27, Q).astype(f16)
    w1c = np.zeros((59, Q), f16)
    w1c[0:27] = w1c27
    w1c[32:59] = w1c27
    # wa: [128, 384]; cols 0-191: dy=0 (rows 0-63) / dy=1 (rows 64-127) taps
    #     cols 192-383: dy=2 taps on rows 64-127
    wa = np.zeros((128, 384), f16)
    for dx in range(3):
        wa[0:64, 64 * dx : 64 * dx + 64] = w2[:, :, 0, dx].T
        wa[64:128, 64 * dx : 64 * dx + 64] = w2[:, :, 1, dx].T
        wa[64:128, 192 + 64 * dx : 256 + 64 * dx] = w2[:, :, 2, dx].T
    # conv3 folded into the cluster dot: 2 mu.(W3 q2 + b3) - |mu|^2
    #   = (2 mu W3).q2 + (2 mu.b3 - |mu|^2)
    w3r = w3.reshape(Q, Q).astype(np.float64)
    mu = cluster_mu.reshape(NC, Q).astype(np.float64)
    m2 = mu @ w3r                                  # (NC, Q) over q2 channels
    mu2 = (2.0 * m2).T.astype(f16)
    mu2b = np.ascontiguousarray(np.tile(mu2, (2, 1)))
    lb4 = np.tile(np.ascontiguousarray(cluster_label.T), (4, 1)).astype(bf16)
    onesb = np.zeros((128, 4), bf16)
    for j in range(4):
        onesb[32 * j : 32 * j + 32, j] = 1
    ebias = 2.0 * (mu @ b3.astype(np.float64)) - np.sum(mu * mu, axis=1)
    nmun = np.tile(ebias, 4).reshape(128, 1).astype(np.float32)
    shared = {
        "w1c": w1c, "wa": wa, "mu2b": mu2b, "lb4": lb4,
        "onesb": onesb,
        "b1c": np.tile(b1, 2).reshape(128, 1).astype(np.float32),
        "b2c": np.tile(b2, 2).reshape(128, 1).astype(np.float32),
        "nmun": nmun,
    }
    return [{"xpad": np.ascontiguousarray(xpad[b]), **shared} for b in range(B)]


def run(inputs, trace=False, **trace_kwargs):
    """Build (cached), run on 8 cores, return (output, BassKernelResults)."""
    if "nc" not in _cache:
        _cache["nc"] = _build()
    in_maps = _prep_inputs(**{k: np.asarray(v) for k, v in inputs.items()})
    res = bass_utils.run_bass_kernel_spmd(
        _cache["nc"], in_maps, core_ids=list(range(B)), trace=trace, **trace_kwargs
    )
    out = np.stack([res.results[b]["res"] for b in range(B)]).astype(np.float32)
    return out, res


def kernel(**inputs):
    out, _ = run(inputs)
    return out

